# revision 68
# baseline (speedup 1.0000x reference)
"""Trainium2 Bass kernel for tanh-attention (nn_Attention_50362786513376).

reference:
  q = (x @ Wq.T) * dk^-0.5 ; k = x @ Wk.T ; v = x        (heads = 8, dk = 64)
  out = tanh(q k^T) v   per (batch, head),  merged back to [b, n, dim]

Sharding: 8 cores = 4 batches x 2 head-halves (4 heads per core).
Host pre-work (free, exact): transpose x[b] -> xT, slice v channels, slice +
scale + transpose weights. Device per core:
  Q^T = WqT.T @ xT, K^T = WkT.T @ xT     (f16, chunk-chased: projections for
                                          n-chunk t4 start as soon as that
                                          xT column block lands in SBUF)
  per head pair p, i-quarter, j-tile: S^T[j,i] = K^T.T Q^T (row-packed pairs)
  tanh on ScalarE PSUM->SBUF (the throughput bottleneck: n^2*h*b/8 elements)
  out^T[d,i] += v[j,:].T @ tanh(S^T)     (col-tiled pair into one packed
                                          PSUM bank, accumulated over j)
Host post-work: out[b,:,half] = outT.T (f16 staged, upcast on host)
"""
import numpy as np

HEADS = 8
DK = 64
B = 4
N = 2048
DIM = 512
SCALE = DK ** (-0.5)
NCORES = 8
HALF = DIM // 2  # 256 channels per core (4 heads)

_built = None
_built_cfg = None
PROJ_DTYPE = "f16"   # x / weights / projection matmuls
ATTN_DTYPE = "f16"   # Q^T/K^T, qk mms
V_DTYPE = "f16"      # tanh output + v operand of the AV mms
TRACE = False
TRACE_KW = {}
WARM_MM = 6    # dummy matmuls to un-throttle the PE HAM during the DMA wait

# VectorE tanh offload: a fraction of the 128 [128,1024] score tiles is
# evaluated on the otherwise-idle DVE with a clamped odd polynomial
# x*(c0 + c1 y + c2 y^2), y = clamp(x)^2 (N(0,1)-weighted lsq fit, rms err
# 1.5e-2 -- output contribution ~0.25 rms vs 6.4 abs budget), freeing
# ScalarE (the bottleneck) of those tiles.
DVE_CLAMP = 2.5
DVE_C = (0.93689209, -0.17860708, 0.01512831)
# per (p,iq) group: j positions evaluated on DVE (never j=0). Group 0 is
# DMA-paced (ScalarE has idle slack there) so offloading it buys nothing;
# the last group ends early so its chains don't push the tail out.
DVE_JS = [(), (5, 11), (5, 11), (5, 11),
          (5, 11), (5, 11), (5, 11), (4, 9)]


def _build():
    from contextlib import ExitStack

    import concourse.tile as tile
    from concourse import bacc, mybir

    F32 = mybir.dt.float32
    DT = {"f32r": mybir.dt.float32r, "f16": mybir.dt.float16,
          "bf16": mybir.dt.bfloat16}
    PROJ_DT = DT[PROJ_DTYPE]
    ATTN_DT = DT[ATTN_DTYPE]
    V_DT = DT[V_DTYPE]
    Tanh = mybir.ActivationFunctionType.Tanh

    nc = bacc.Bacc("TRN2", target_bir_lowering=False, debug=False,
                   num_devices=NCORES)
    # All inputs are host-packed into the exact SBUF images so every DMA is
    # contiguous 2D with multi-KB lines:
    #   xT_img [128, t4-major (t4, ct, 512)], xv_img [128, (j, 256)],
    #   w*_img [128, p-major (p, ct, 128)]
    xT_ap = nc.dram_tensor("xT", [128, 4 * N], PROJ_DT,
                           kind="ExternalInput").ap()
    xv_ap = nc.dram_tensor("xv", [128, (N // 128) * HALF], V_DT,
                           kind="ExternalInput").ap()
    wqT_ap = nc.dram_tensor("wqT", [128, 4 * HALF], PROJ_DT,
                            kind="ExternalInput").ap()
    wkT_ap = nc.dram_tensor("wkT", [128, 4 * HALF], PROJ_DT,
                            kind="ExternalInput").ap()
    outT_ap = nc.dram_tensor("outT", [HALF, N], V_DT,
                             kind="ExternalOutput").ap()

    NT = N // 512          # 4 t-chunks of 512
    NJ = N // 128          # 16 j-tiles

    with tile.TileContext(nc) as tc:
        with ExitStack() as ctx:
            const = ctx.enter_context(tc.tile_pool(name="const", bufs=1))
            qk_pool = ctx.enter_context(tc.tile_pool(name="qk", bufs=1))
            tanh_pool = ctx.enter_context(tc.tile_pool(name="tanh", bufs=6))
            stg_pool = ctx.enter_context(tc.tile_pool(name="stg", bufs=4))
            dve_pool = ctx.enter_context(tc.tile_pool(name="dve", bufs=2))
            dve_T = ctx.enter_context(tc.tile_pool(name="dveT", bufs=4))

            # ---- input DMAs: all contiguous 2D copies of host-packed
            # SBUF images, chunk-chased across queues ----
            # SBUF layouts: xT_sb col = t4*2048 + ct*512 + c (t4-major);
            # w_sb col = p*512 + ct*128 + c (p-major); xv_sb col = j*256 + c
            xT_sb = const.tile([128, 4 * N], PROJ_DT)
            wq_sb = const.tile([128, 4 * HALF], PROJ_DT)
            wk_sb = const.tile([128, 4 * HALF], PROJ_DT)
            xv_sb = const.tile([128, NJ * HALF], V_DT)

            # weights for head-pair 0 at the head of the two fast queues;
            # the first xT chunk (t4=0) goes ct-granular so the first
            # projection's matmuls can chase individual 128KB pieces.
            # All three DMA queues are load-balanced (~1.1-1.25MB each) and
            # ordered by when the stream needs each piece.
            # the critical first 0.75MB (p0 weights + xT t4=0) split across
            # all three queues; the scalar queue stalls ~2us around t=10us
            # so nothing chase-critical beyond wq rides it
            nc.sync.dma_start(wk_sb[:, 0:512], wkT_ap[:, 0:512])
            nc.scalar.dma_start(wq_sb[:, 0:512], wqT_ap[:, 0:512])
            nc.gpsimd.dma_start(xT_sb[:, 1024:1536], xT_ap[:, 1024:1536])
            nc.sync.dma_start(xT_sb[:, 0:512], xT_ap[:, 0:512])
            nc.gpsimd.dma_start(xT_sb[:, 1536:2048], xT_ap[:, 1536:2048])
            nc.sync.dma_start(xT_sb[:, 512:1024], xT_ap[:, 512:1024])
            # remaining chunks, ordered by stream need-time
            nc.gpsimd.dma_start(xT_sb[:, 2048:4096], xT_ap[:, 2048:4096])
            nc.scalar.dma_start(xv_sb[:, 0:1024], xv_ap[:, 0:1024])
            nc.sync.dma_start(xT_sb[:, 4096:6144], xT_ap[:, 4096:6144])
            nc.scalar.dma_start(xT_sb[:, 6144:8192], xT_ap[:, 6144:8192])
            nc.sync.dma_start(xv_sb[:, 1024:2048], xv_ap[:, 1024:2048])
            nc.gpsimd.dma_start(wk_sb[:, 512:1024], wkT_ap[:, 512:1024])
            nc.scalar.dma_start(xv_sb[:, 2048:3072], xv_ap[:, 2048:3072])
            nc.sync.dma_start(xv_sb[:, 3072:4096], xv_ap[:, 3072:4096])
            nc.scalar.dma_start(wq_sb[:, 512:1024], wqT_ap[:, 512:1024])

            def proj_pair_chase():
                # first K/Q projections: matmuls interleaved in chunk-arrival
                # order (ct 0,2 land first), casts split ScalarE/VectorE so
                # they run in parallel
                psk = ps_S.tile([128, 1024], F32, tag="S", name="pk")[:, 0:512]
                psq = ps_S.tile([128, 1024], F32, tag="S", name="pq")[:, 0:512]
                for k, ct in enumerate((2, 0, 3, 1)):
                    for ps, w_sb in ((psk, wk_sb), (psq, wq_sb)):
                        nc.tensor.matmul(
                            ps[:], w_sb[:, ct * 128:ct * 128 + 128],
                            xT_sb[:, ct * 512:ct * 512 + 512],
                            start=(k == 0), stop=(k == 3))
                nc.scalar.copy(KT[0][:, 0:512], psk[:])
                nc.vector.tensor_copy(QT[0][:, 0:512], psq[:])


            # ---- PSUM pools ----
            # ps_S: 3 x [128,1024] (6 banks); ps_acc: 2 x [128,512] (2 banks,
            # both AV parities col-tiled into one bank at rows 0-63 / 64-127)
            QT = [qk_pool.tile([128, N], ATTN_DT, tag=f"qt{p}", name=f"qt{p}")
                  for p in range(2)]
            KT = [qk_pool.tile([128, N], ATTN_DT, tag=f"kt{p}", name=f"kt{p}")
                  for p in range(2)]
            ps_S = ctx.enter_context(
                tc.tile_pool(name="ps_S", bufs=3, space="PSUM"))
            ps_acc = ctx.enter_context(
                tc.tile_pool(name="ps_acc", bufs=1, space="PSUM"))
            ps_proj = ctx.enter_context(
                tc.tile_pool(name="ps_proj", bufs=1, space="PSUM"))

            # ---- PE warm-up: dummy matmuls during the input-DMA wait so
            # HAM un-throttles before the first projection (sized to end
            # roughly when the first xT chunk lands, not after)
            warm_src = const.tile([128, 512], PROJ_DT, name="warm_src")
            nc.vector.memset(warm_src[:], 0)
            warm = ps_proj.tile([128, 512], F32, tag="proj", name="warm")
            for _ in range(WARM_MM):
                nc.tensor.matmul(warm[:, 0:256], warm_src[:, 0:128],
                                 warm_src[:, 0:256], start=True, stop=True)

            # ---- projections (interleaved into the attention stream) ----
            # PSUM borrows rotating ps_S slots.
            from concourse.tile_rust import add_dep_helper

            class ProjSmear:
                """In-stream projection spread one matmul per stream tile:
                a 4-matmul burst exceeds ScalarE's S-buffer lookahead and
                stalls the tanh stream; single-matmul injections don't.
                Each matmul is order-pinned behind the current QK so the
                scheduler can't hoist it."""

                def __init__(self):
                    self.queue = []
                    self.active = None
                    self.ct = 0

                def add(self, dst, w_sb, p, t4):
                    self.queue.append((dst, w_sb, p, t4))

                def step(self, after):
                    if self.active is None:
                        if not self.queue:
                            return
                        self.active = self.queue.pop(0)
                        self.ct = 0
                        self.ps = ps_proj.tile([128, 512], F32, tag="proj",
                                               name="proj_ps")
                    dst, w_sb, p, t4 = self.active
                    ct = self.ct
                    lhsT = w_sb[:, p * 512 + ct * 128:p * 512 + ct * 128 + 128]
                    rhs = xT_sb[:, t4 * 2048 + ct * 512:
                                t4 * 2048 + ct * 512 + 512]
                    mm = nc.tensor.matmul(self.ps[:], lhsT, rhs,
                                          start=(ct == 0), stop=(ct == 3))
                    if after is not None:
                        add_dep_helper(mm.ins, after.ins, sync=False,
                                       reason="keep proj behind the stream")
                    self.ct += 1
                    if self.ct == 4:
                        nc.vector.tensor_copy(
                            dst[p][:, t4 * 512:(t4 + 1) * 512], self.ps[:])
                        self.active = None

            def qk_pair(p, iq, j):
                i0 = iq * 512
                S = ps_S.tile([128, 1024], F32, tag="S", name="S")
                # row-packed pair: head parity 0 on PE rows 0-63,
                # parity 1 on rows 64-127
                qk0 = nc.tensor.matmul(
                    S[:, 0:512],
                    KT[p][0:64, j * 128:(j + 1) * 128],
                    QT[p][0:64, i0:i0 + 512],
                    start=True, stop=True, tile_position=(0, 0))
                nc.tensor.matmul(
                    S[:, 512:1024],
                    KT[p][64:128, j * 128:(j + 1) * 128],
                    QT[p][64:128, i0:i0 + 512],
                    start=True, stop=True, tile_position=(64, 0))
                return S, qk0

            def av_pair(p, j, acc, T, start, stop):
                # AV pair col-tiled: par0 -> acc rows 0-63 (PE cols 0-63),
                # par1 -> acc rows 64-127 (PE cols 64-127); concurrent on
                # disjoint col groups
                for par in range(2):
                    lh = 2 * p + par
                    v = xv_sb[:, j * HALF + lh * 64:j * HALF + lh * 64 + 64]
                    nc.tensor.matmul(
                        acc[par * 64:(par + 1) * 64, :],
                        v,
                        T[:, par * 512:(par + 1) * 512],
                        start=start, stop=stop,
                        tile_position=(0, par * 64))

            def attn_tile(p, iq, j, acc, start=False, stop=False,
                          defer=None):
                S, qk0 = qk_pair(p, iq, j)
                T = tanh_pool.tile([128, 1024], V_DT, tag="T", name="T")
                nc.scalar.activation(T[:], S[:], Tanh)
                if defer is None:
                    av_pair(p, j, acc, T, start=start, stop=stop)
                else:
                    defer.append((j, T))
                return qk0

            Mul, Add = mybir.AluOpType.mult, mybir.AluOpType.add
            Min, Max = mybir.AluOpType.min, mybir.AluOpType.max
            c0, c1, c2 = DVE_C

            def dve_tile(p, iq, j):
                # tanh via clamped odd polynomial, entirely on VectorE:
                # xc = clamp(S); y = xc^2; T = (c0 + y(c1 + c2 y)) * xc
                S, qk0 = qk_pair(p, iq, j)
                xc = dve_pool.tile([128, 1024], V_DT, tag="xc", name="xc")
                y = dve_pool.tile([128, 1024], V_DT, tag="y", name="y")
                a = dve_pool.tile([128, 1024], V_DT, tag="a", name="a")
                b = dve_pool.tile([128, 1024], V_DT, tag="b", name="b")
                T = dve_T.tile([128, 1024], V_DT, tag="Tdve", name="Tdve")
                nc.vector.tensor_scalar(xc[:], S[:], DVE_CLAMP, -DVE_CLAMP,
                                        Min, Max)
                nc.vector.tensor_tensor(y[:], xc[:], xc[:], Mul)
                nc.vector.tensor_scalar(a[:], y[:], c2, c1, Mul, Add)
                nc.vector.tensor_tensor(b[:], a[:], y[:], Mul)
                nc.vector.tensor_scalar(a[:], b[:], 1.0, c0, Mul, Add)
                nc.vector.tensor_tensor(T[:], a[:], xc[:], Mul)
                return T, qk0

            def store_acc(p, iq, acc):
                st = stg_pool.tile([128, 512], V_DT, tag="stg", name="stg")
                nc.vector.tensor_copy(st[:], acc[:])
                nc.gpsimd.dma_start(
                    outT_ap[p * 128:(p + 1) * 128, iq * 512:(iq + 1) * 512],
                    st[:])

            # ---- stream schedule ----
            # (p0, iq0): j-tiles chase the xT chunks; KT0 chunk t4 is
            # projected right before the j-tiles that read it (j = 4*t4 ..).
            # (p0, iq0) chunk-chases xT; later groups interleave the
            # remaining projections (pinned behind the stream) and the DVE
            # tanh tiles, whose AV matmuls are deferred to the group end.
            p1_proj = [(KT, wk_sb, 1, 0), (QT, wq_sb, 1, 0),
                       (KT, wk_sb, 1, 1), (QT, wq_sb, 1, 1),
                       (KT, wk_sb, 1, 2), (QT, wq_sb, 1, 2),
                       (KT, wk_sb, 1, 3), (QT, wq_sb, 1, 3)]

            # Cross-group pipelining: a group's DVE-tile AVs (which wait on
            # the polynomial chains) and its acc store are flushed inside
            # the NEXT group after its first 3 QK/ACT tiles, so a late
            # chain never blocks the Tensor FIFO ahead of ready QK work.
            # The next group's own early AVs defer until after the
            # handover (the single acc bank is released by the store).
            pending = []   # [(p, acc, [(j, T), ...])] from the prior group

            def flush_pending(last=False):
                while pending:
                    pp, pacc, piq, items = pending.pop(0)
                    for k, (j, T) in enumerate(items):
                        av_pair(pp, j, pacc, T, start=False,
                                stop=(k == len(items) - 1))
                    store_acc(pp, piq, pacc)

            smear = ProjSmear()

            def group(g, p, iq):
                dve_js = DVE_JS[g]
                acc = ps_acc.tile([128, 512], F32, tag="acc", name="acc")
                deferred = []
                early = []
                for j in range(NJ):
                    if j in dve_js:
                        T, qk = dve_tile(p, iq, j)
                        deferred.append((j, T))
                    else:
                        qk = attn_tile(p, iq, j, acc, start=(j == 0),
                                       stop=(j == NJ - 1 and not deferred),
                                       defer=early if j < 3 else None)
                    if j == 2:
                        flush_pending()
                        for k, (je, T) in enumerate(early):
                            av_pair(p, je, acc, T, start=(k == 0),
                                    stop=False)
                        early = None
                    # enqueue this group's projections, one matmul per tile
                    if p == 0 and iq == 0:
                        if j % 4 == 0 and j < 12:
                            smear.add(KT, wk_sb, 0, j // 4 + 1)
                        elif j == 11:
                            smear.add(QT, wq_sb, 0, 1)
                    elif p == 0 and iq < 3:
                        if j == 10:
                            smear.add(QT, wq_sb, 0, iq + 1)
                        elif j in (1, 5) and p1_proj:
                            smear.add(*p1_proj.pop(0))
                    elif p1_proj and j in (1, 5, 10):
                        smear.add(*p1_proj.pop(0))
                    smear.step(qk)
                pending.append((p, acc, iq, deferred))

            for g, (p, iq) in enumerate((p, iq) for p in range(2)
                                        for iq in range(4)):
                if g == 0:
                    proj_pair_chase()
                group(g, p, iq)
            flush_pending(last=True)

    nc.compile()
    return nc


def _get_built():
    global _built, _built_cfg
    cfg = (PROJ_DTYPE, ATTN_DTYPE, V_DTYPE)
    if _built is None or _built_cfg != cfg:
        _built = _build()
        _built_cfg = cfg
    return _built


def kernel(x, Wq, Wk):
    from concourse.bass_utils import run_bass_kernel_spmd

    x = np.asarray(x, dtype=np.float32)
    Wq = np.asarray(Wq, dtype=np.float32)
    Wk = np.asarray(Wk, dtype=np.float32)

    proj_np = np.float16
    v_np = np.float16

    def pack_xT(xb):
        # [512, 2048] -> img[part, t4*2048 + ct*512 + c]
        xT = xb.T.astype(proj_np)
        return np.ascontiguousarray(
            xT.reshape(4, 128, 4, 512).transpose(1, 2, 0, 3)
            .reshape(128, 8192))

    def pack_w(wT):
        # [512, 256] -> img[part, p*512 + ct*128 + c]
        return np.ascontiguousarray(
            wT.reshape(4, 128, 2, 128).transpose(1, 2, 0, 3)
            .reshape(128, 1024))

    def pack_xv(xvb):
        # [2048, 256] -> img[part, j*256 + c]
        return np.ascontiguousarray(
            xvb.reshape(16, 128, 256).transpose(1, 0, 2).reshape(128, 4096))

    nc = _get_built()
    in_maps = []
    for c in range(NCORES):
        b, half = c // 2, c % 2
        sl = slice(half * HALF, (half + 1) * HALF)
        in_maps.append({
            "xT": pack_xT(x[b]),
            "xv": pack_xv(x[b][:, sl].astype(v_np)),
            "wqT": pack_w((SCALE * Wq[sl, :]).T.astype(proj_np)),
            "wkT": pack_w(Wk[sl, :].T.astype(proj_np)),
        })
    try:
        res = run_bass_kernel_spmd(nc, in_maps, core_ids=list(range(NCORES)),
                                   trace=TRACE, **TRACE_KW)
    except Exception:
        # transient device wedge (NRT_EXEC_UNIT_UNRECOVERABLE) recovers on
        # retry; one attempt is enough in practice
        import time as _time
        _time.sleep(2.0)
        res = run_bass_kernel_spmd(nc, in_maps, core_ids=list(range(NCORES)),
                                   trace=TRACE, **TRACE_KW)
    out = np.empty((B, N, DIM), np.float32)
    for c in range(NCORES):
        b, half = c // 2, c % 2
        out[b, :, half * HALF:(half + 1) * HALF] = \
            res.results[c]["outT"].T.astype(np.float32)
    if TRACE:
        kernel.last_results = res
    return out
